# revision 9
# baseline (speedup 1.0000x reference)
"""Trainium2 Bass kernel for batched sparse-attention MLP scoring.

B=2048 samples sharded 256/core across 8 cores (pure data parallel).
Per sample: score[t] = MLP(concat([q, k_t, q-k_t, q*k_t])), masked softmax
over t, output = sum_t softmax[t] * V[t].

Math folding (exact):
  emb @ W1 = q@(W1a+W1c) + k@(W1b-W1c) + (q*k)@W1d
so per-token L1 = W1kq.T @ [kt; q*kt] (K=128) with per-sample bias
C_b = q_b@(W1a+W1c)+b1 applied via the ScalarE activation bias port.
bo is softmax-shift-invariant and dropped; mask folds to exp(score)*mask;
normalization is applied after the V-contraction (out = u/Z).

Scores are produced transposed (t on partitions, sample on free) so the
softmax sum is a ones-matmul and the V contraction consumes score columns
directly; per-sample weights/biases always ride partition-indexed operands.

Host-side prep (layout only): K transposed to [D,T] and cast bf16, V packed
two-samples-wide for >=512B/partition DMAs, mask transposed, weights bf16,
C precomputed (tiny: B x 4D x H1).
"""

import sys

sys.path.insert(0, "/opt/trn_rl_repo")

from contextlib import ExitStack

import numpy as np
import ml_dtypes

import concourse.bass as bass
import concourse.bacc as bacc
import concourse.tile as tile
import concourse.mybir as mybir

BF16 = mybir.dt.bfloat16
F32 = mybir.dt.float32
AF = mybir.ActivationFunctionType
ALU = mybir.AluOpType
AX = mybir.AxisListType

B, T, D, H1, H2 = 2048, 200, 64, 128, 64
NCORE = 8
BC = B // NCORE      # 256 samples per core
BLK = 128            # samples per softmax block
NBLK = BC // BLK     # 2
NPAIR = BLK // 2     # 64 pairs per block
T0 = 128             # first t chunk
T1 = T - T0          # 72


def build_nc():
    nc = bacc.Bacc("TRN2", target_bir_lowering=False, debug=False)
    ktcat = nc.dram_tensor("ktcat", [BC // 2, D, 2 * T], BF16, kind="ExternalInput")
    vp = nc.dram_tensor("vp", [BC // 2, T, 2 * D], F32, kind="ExternalInput")
    masktr = nc.dram_tensor("masktr", [T, BC], F32, kind="ExternalInput")
    ct = nc.dram_tensor("ct", [H1, BC], F32, kind="ExternalInput")
    qlt = nc.dram_tensor("qlt", [D, BC], F32, kind="ExternalInput")
    w1kq = nc.dram_tensor("w1kq", [2 * D, H1], BF16, kind="ExternalInput")
    w2t = nc.dram_tensor("w2t", [H1, H2], BF16, kind="ExternalInput")
    wop = nc.dram_tensor("wop", [H1, 1], BF16, kind="ExternalInput")
    b2p = nc.dram_tensor("b2p", [H1, 1], F32, kind="ExternalInput")
    ident = nc.dram_tensor("ident", [128, 128], F32, kind="ExternalInput")
    onesd = nc.dram_tensor("onesd", [128, 1], F32, kind="ExternalInput")
    outd = nc.dram_tensor("out", [BC, D], F32, kind="ExternalOutput")

    with tile.TileContext(nc) as tc, ExitStack() as ctx:
        pers = ctx.enter_context(tc.tile_pool(name="pers", bufs=1))
        spool = ctx.enter_context(tc.tile_pool(name="s", bufs=6))
        h1p = ctx.enter_context(tc.tile_pool(name="h1", bufs=10))
        h2p = ctx.enter_context(tc.tile_pool(name="h2", bufs=3))
        ep = ctx.enter_context(tc.tile_pool(name="e", bufs=2))
        vap = ctx.enter_context(tc.tile_pool(name="va", bufs=BLK + 8))
        vbp = ctx.enter_context(tc.tile_pool(name="vb", bufs=BLK + 8))
        z1pool = ctx.enter_context(tc.tile_pool(name="z1", bufs=2, space="PSUM"))
        z2pool = ctx.enter_context(tc.tile_pool(name="z2", bufs=2, space="PSUM"))
        scpool = ctx.enter_context(tc.tile_pool(name="scp", bufs=1, space="PSUM"))
        mcpool = ctx.enter_context(tc.tile_pool(name="mc", bufs=1, space="PSUM"))

        W1bc = pers.tile([D, H1], BF16)
        nc.sync.dma_start(W1bc[:], w1kq[0:D, :])
        W1d = pers.tile([D, H1], BF16)
        nc.sync.dma_start(W1d[:], w1kq[D:2 * D, :])
        W2 = pers.tile([H1, H2], BF16)
        nc.sync.dma_start(W2[:], w2t[:])
        WO = pers.tile([H1, 1], BF16)
        nc.sync.dma_start(WO[:], wop[:])
        CT = pers.tile([H1, BC], F32)
        nc.sync.dma_start(CT[:], ct[:])
        QL = pers.tile([D, BC], F32)
        nc.sync.dma_start(QL[:], qlt[:])
        B2 = pers.tile([H1, 1], F32)
        nc.sync.dma_start(B2[:], b2p[:])
        ID = pers.tile([128, 128], F32)
        nc.sync.dma_start(ID[:], ident[:])
        ON = pers.tile([128, 1], F32)
        nc.sync.dma_start(ON[:], onesd[:])

        for blk in range(NBLK):
            s0 = blk * BLK
            mk = ep.tile([128, 2 * BLK], F32, tag="mask")
            nc.sync.dma_start(mk[:, 0:BLK], masktr[0:T0, s0:s0 + BLK])
            nc.sync.dma_start(mk[0:T1, BLK:2 * BLK], masktr[T0:T, s0:s0 + BLK])

            # scT columns: col s = scores of sample s for t in chunk
            scT = scpool.tile([128, 2 * BLK], F32, tag="sc")
            h1_tiles = {}
            vtiles = {}
            for p in range(NPAIR):
                pg = blk * NPAIR + p
                sa, sb = s0 + 2 * p, s0 + 2 * p + 1
                Skt = spool.tile([D, 2 * T], BF16, tag="Skt")
                nc.sync.dma_start(Skt[:], ktcat[pg])
                Sqk = spool.tile([D, 2 * T], BF16, tag="Sqk")
                nc.vector.tensor_scalar(
                    Sqk[:, 0:T], Skt[:, 0:T],
                    QL[0:D, sa:sa + 1], None, ALU.mult)
                nc.vector.tensor_scalar(
                    Sqk[:, T:2 * T], Skt[:, T:2 * T],
                    QL[0:D, sb:sb + 1], None, ALU.mult)

                va = vap.tile([T0, 2 * D], F32, tag="va")
                vb = vbp.tile([T1, 2 * D], F32, tag="vb")
                nc.sync.dma_start(va[:], vp[pg, 0:T0, :])
                nc.sync.dma_start(vb[:], vp[pg, T0:T, :])
                vtiles[p] = (va, vb)

                z1 = z1pool.tile([128, 2 * T], F32, tag="z1")
                nc.tensor.matmul(z1[:, 0:T], W1bc[:], Skt[:, 0:T], start=True, stop=False)
                nc.tensor.matmul(z1[:, 0:T], W1d[:], Sqk[:, 0:T], start=False, stop=True)
                nc.tensor.matmul(z1[:, T:2 * T], W1bc[:], Skt[:, T:2 * T], start=True, stop=False)
                nc.tensor.matmul(z1[:, T:2 * T], W1d[:], Sqk[:, T:2 * T], start=False, stop=True)
                h1a = h1p.tile([H1, T], BF16, tag="h1")
                h1b = h1p.tile([H1, T], BF16, tag="h1")
                nc.scalar.activation(h1a[:], z1[:, 0:T], AF.Relu, bias=CT[:, sa:sa + 1])
                nc.scalar.activation(h1b[:], z1[:, T:2 * T], AF.Relu, bias=CT[:, sb:sb + 1])
                h1_tiles[2 * p] = h1a
                h1_tiles[2 * p + 1] = h1b

                if p % 2 == 1:
                    g = p // 2
                    z2 = z2pool.tile([128, 2 * T], F32, tag="z2")
                    ha = h1_tiles.pop(2 * p - 2)
                    hb = h1_tiles.pop(2 * p - 1)
                    hc = h1_tiles.pop(2 * p)
                    hd = h1_tiles.pop(2 * p + 1)
                    nc.tensor.matmul(z2[0:H2, 0:T], W2[:], ha[:], start=True, stop=True)
                    nc.tensor.matmul(z2[H2:128, 0:T], W2[:], hb[:], start=True, stop=True)
                    nc.tensor.matmul(z2[0:H2, T:2 * T], W2[:], hc[:], start=True, stop=True)
                    nc.tensor.matmul(z2[H2:128, T:2 * T], W2[:], hd[:], start=True, stop=True)
                    h2 = h2p.tile([128, 2 * T], BF16, tag="h2")
                    nc.scalar.activation(h2[:], z2[:], AF.Relu, bias=B2[:, 0:1])
                    # transposed scores: for sample s, lhsT = h2 slice
                    # [64 feats, t-cols]; out = scT[:, col]
                    for j2 in range(4):
                        col = 4 * g + j2
                        rsl = slice(64 * (j2 % 2), 64 * (j2 % 2) + 64)
                        cbase = T * (j2 // 2)
                        nc.tensor.matmul(scT[0:T0, col:col + 1],
                                         h2[rsl, cbase:cbase + T0], WO[rsl, :],
                                         start=True, stop=True)
                        nc.tensor.matmul(scT[0:T1, BLK + col:BLK + col + 1],
                                         h2[rsl, cbase + T0:cbase + T], WO[rsl, :],
                                         start=True, stop=True)

            # block epilogue (t-major layout): exp, mask, Z, V-contraction
            E = ep.tile([128, 2 * BLK], F32, tag="E")
            nc.scalar.activation(E[:, 0:BLK], scT[:, 0:BLK], AF.Exp)
            nc.scalar.activation(E[0:T1, BLK:2 * BLK], scT[0:T1, BLK:2 * BLK], AF.Exp)
            nc.vector.tensor_mul(E[:, 0:BLK], E[:, 0:BLK], mk[:, 0:BLK])
            nc.vector.tensor_mul(E[0:T1, BLK:2 * BLK], E[0:T1, BLK:2 * BLK],
                                 mk[0:T1, BLK:2 * BLK])

            Zp = mcpool.tile([BLK, 1], F32, tag="Z")
            nc.tensor.matmul(Zp[:], E[:, 0:BLK], ON[:], start=True, stop=False)
            nc.tensor.matmul(Zp[:], E[0:T1, BLK:2 * BLK], ON[0:T1, :],
                             start=False, stop=True)
            R = ep.tile([BLK, 1], F32, tag="R")
            nc.vector.reciprocal(R[:], Zp[:])

            u = mcpool.tile([H2, 2 * BLK], F32, tag="u")
            for p in range(NPAIR):
                va, vb = vtiles.pop(p)
                for j in range(2):
                    s = 2 * p + j
                    dcol = slice(j * D, (j + 1) * D)
                    nc.tensor.matmul(u[:, s:s + 1], va[:, dcol],
                                     E[0:T0, s:s + 1], start=True, stop=True)
                    nc.tensor.matmul(u[:, BLK + s:BLK + s + 1], vb[:, dcol],
                                     E[0:T1, BLK + s:BLK + s + 1],
                                     start=True, stop=True)

            ub = ep.tile([H2, BLK], F32, tag="ub")
            nc.vector.tensor_copy(ub[:], u[:, BLK:2 * BLK])
            us = ep.tile([H2, BLK], F32, tag="us")
            nc.vector.tensor_add(us[:], u[:, 0:BLK], ub[:])
            oT = mcpool.tile([BLK, H2], F32, tag="oT")
            nc.tensor.transpose(oT[:], us[:], ID[0:H2, 0:H2])
            oS = ep.tile([BLK, H2], F32, tag="oS")
            nc.vector.tensor_scalar(oS[:], oT[:], R[:, 0:1], None, ALU.mult)
            nc.sync.dma_start(outd[s0:s0 + BLK, :], oS[:])
    nc.compile()
    return nc


def host_prep(query, key, value, mask, W1, b1, W2, b2, Wo, bo):
    bf16 = ml_dtypes.bfloat16
    f32 = np.float32
    query = np.asarray(query, f32)
    key = np.asarray(key, f32)
    value = np.asarray(value, f32)
    W1 = np.asarray(W1, f32)

    W1a, W1b, W1c, W1d = W1[0:64], W1[64:128], W1[128:192], W1[192:256]
    w1kq = np.ascontiguousarray(np.concatenate([W1b - W1c, W1d], 0)).astype(bf16)
    C = (query.astype(np.float64) @ (W1a + W1c).astype(np.float64)
         + np.asarray(b1, np.float64)).astype(f32)          # [B, H1]
    w2b = np.ascontiguousarray(np.asarray(W2, f32)).astype(bf16)
    wob = np.ascontiguousarray(np.concatenate([np.asarray(Wo, f32)] * 2, 0)).astype(bf16)  # [H1,1]
    b2pair = np.concatenate([np.asarray(b2, f32), np.asarray(b2, f32)])[:, None]
    ident = np.eye(128, dtype=f32)
    ones = np.ones((128, 1), f32)

    in_maps = []
    for c in range(NCORE):
        sl = slice(c * BC, (c + 1) * BC)
        kt = key[sl].transpose(0, 2, 1)                       # [BC, D, T]
        ktc = np.ascontiguousarray(
            kt.reshape(BC // 2, 2, D, T).transpose(0, 2, 1, 3)
        ).reshape(BC // 2, D, 2 * T).astype(bf16)
        vpp = np.ascontiguousarray(
            value[sl].reshape(BC // 2, 2, T, D).transpose(0, 2, 1, 3)
        ).reshape(BC // 2, T, 2 * D)
        mtr = np.ascontiguousarray(mask[sl].T).astype(f32)    # [T, BC]
        ctc = np.ascontiguousarray(C[sl].T)                   # [H1, BC]
        ql = np.ascontiguousarray(query[sl].T)                # [D, BC]
        in_maps.append({
            "ktcat": ktc, "vp": vpp, "masktr": mtr, "ct": ctc, "qlt": ql,
            "w1kq": w1kq, "w2t": w2b, "wop": wob, "b2p": b2pair,
            "ident": ident, "onesd": ones,
        })
    return in_maps


_NC = None


def kernel(query, key, value, mask, W1, b1, W2, b2, Wo, bo):
    global _NC
    from concourse.bass_utils import run_bass_kernel_spmd
    in_maps = host_prep(query, key, value, mask, W1, b1, W2, b2, Wo, bo)
    if _NC is None:
        _NC = build_nc()
    res = run_bass_kernel_spmd(_NC, in_maps, list(range(NCORE)))
    outs = [np.asarray(res.results[i]["out"], np.float32) for i in range(NCORE)]
    return np.concatenate(outs, 0)
